# revision 39
# baseline (speedup 1.0000x reference)
"""Self-contained Trainium2 Bass kernel for nn_DbrxBlock_40492951667588.

DBRX block: LN1 -> GQA attention (RoPE, causal) -> residual+LN2 -> top-2/8 MoE.
8 NeuronCores, two SPMD launches, all matmuls in bf16:
  launch 1: attention sharded by (batch, kv-head) — core r owns batch r//4,
            kv-head r%4 (4 q heads). Scores computed transposed [k, q] so
            PV needs no PE transposes; causal block-skipping; softmax
            normalization via batched fast-approx reciprocal + gpsimd
            partition broadcast (PE does no ones-matmuls).
  host:     LN1 (pre-launch), partial-sum reduce + residual + LN2 + router
            softmax/top-2 + capacity-padded expert dispatch (between launches).
  launch 2: expert-parallel MoE (core e owns expert e).
"""
import numpy as np
import ml_dtypes
import concourse.bacc as bacc
import concourse.bass as bass
import concourse.mybir as mybir
import concourse.tile as tile
from concourse.bass_utils import run_bass_kernel_spmd

F32 = mybir.dt.float32
BF16 = mybir.dt.bfloat16
AF = mybir.ActivationFunctionType
BF = ml_dtypes.bfloat16

B, S, D = 2, 1024, 2048
DT = D // 128            # 16 d-tiles
NH, KVH, HD = 16, 4, 128
GQ = NH // KVH           # 4 q heads per kv head
NKT = S // 128           # 8 token tiles
EPS = 1e-5
E, K_TOP = 8, 2
MD, MF = 2048, 2048
DT_, FT = MD // 128, MF // 128


# ======================= attention launch =======================

def build_attn(n_cores=8):
    nc = bacc.Bacc("TRN2", target_bir_lowering=False, debug=False,
                   num_devices=n_cores)
    xln = nc.dram_tensor("xln", [DT, 128, S], BF16, kind="ExternalInput").ap()
    wk1 = nc.dram_tensor("wk1", [128, DT, 128], BF16, kind="ExternalInput").ap()
    wv1 = nc.dram_tensor("wv1", [128, DT, 128], BF16, kind="ExternalInput").ap()
    wq4 = nc.dram_tensor("wq4", [128, GQ * DT, 128], BF16,
                         kind="ExternalInput").ap()
    wo4 = nc.dram_tensor("wo4", [128, GQ * DT, 128], BF16,
                         kind="ExternalInput").ap()
    cosk = nc.dram_tensor("cosk", [128, S], BF16, kind="ExternalInput").ap()
    sink = nc.dram_tensor("sink", [128, S], BF16, kind="ExternalInput").ap()
    trilm = nc.dram_tensor("trilm", [128, 128], BF16, kind="ExternalInput").ap()
    onesc = nc.dram_tensor("onesc", [128, 1], BF16, kind="ExternalInput").ap()
    onesr = nc.dram_tensor("onesr", [1, 128], BF16, kind="ExternalInput").ap()
    po = nc.dram_tensor("po", [DT, 128, S], BF16, kind="ExternalOutput").ap()

    HQ = [(0, 512), (512, 512)]  # S-span psum chunks

    with tile.TileContext(nc) as tc:
        with (
            tc.tile_pool(name="cst", bufs=1) as cst,
            tc.tile_pool(name="big", bufs=1) as big,
            tc.tile_pool(name="wts", bufs=1) as wts,
            tc.tile_pool(name="rp", bufs=1) as rp,
            tc.tile_pool(name="nrm", bufs=1) as nrm,
            tc.tile_pool(name="pop", bufs=3) as pop,
        ):
            # DMA issue order tuned for the pass-1 program order (K d0, q0 d0,
            # q1 d0, K d1, ...): wk first, then q-head weights 0/1, then the
            # x stream; cos/sin mid-stream (rope at end of pass 1); wq heads
            # 2/3 before pass 2; wv before pass 3; wo last (outproj is late).
            xln_p = [big.tile([128, 2, S], BF16, name=f"xlnp{i}")
                     for i in range(DT // 2)]

            def dma_x(d):
                nc.sync.dma_start(out=xln_p[d // 2][:, d % 2, :], in_=xln[d])

            # HAM warm-up: zero matmuls during the initial DMA wait keep the
            # PE activity window busy so real matmuls start at full clock.
            # They write the psk0-tagged psum slot; the first real K matmul
            # (start=True) resets it.
            zt = cst.tile([128, 512], BF16)
            nc.gpsimd.memset(zt[:], 0.0)

            # first tiles go out on four different engine queues so HWDGE
            # descriptor generation runs in parallel during the cold start
            wk_sb = wts.tile([128, DT, 128], BF16)
            nc.scalar.dma_start(out=wk_sb[:, :DT // 2, :],
                                in_=wk1[:, :DT // 2, :])
            nc.sync.dma_start(out=xln_p[0][:, 0, :], in_=xln[0])
            wq_sb = wts.tile([128, GQ * DT, 128], BF16)
            nc.gpsimd.dma_start(out=wq_sb[:, :DT, :], in_=wq4[:, :DT, :])
            nc.gpsimd.dma_start(out=wq_sb[:, DT:2 * DT, :],
                                in_=wq4[:, DT:2 * DT, :])
            nc.scalar.dma_start(out=xln_p[0][:, 1, :], in_=xln[1])
            for d in range(2, 4):
                dma_x(d)
            nc.sync.dma_start(out=wk_sb[:, DT // 2:, :], in_=wk1[:, DT // 2:, :])
            for d in range(4, 6):
                dma_x(d)
            cos_sb = cst.tile([128, S], BF16)
            nc.sync.dma_start(out=cos_sb[:], in_=cosk[:])
            sin_sb = cst.tile([128, S], BF16)
            nc.sync.dma_start(out=sin_sb[:], in_=sink[:])
            for d in range(6, DT):
                dma_x(d)
            nc.sync.dma_start(out=wq_sb[:, 2 * DT:, :], in_=wq4[:, 2 * DT:, :])
            wv_sb = wts.tile([128, DT, 128], BF16)
            nc.sync.dma_start(out=wv_sb[:], in_=wv1[:])
            trilm_sb = cst.tile([128, 128], BF16)
            nc.sync.dma_start(out=trilm_sb[:], in_=trilm[:])
            onesc_sb = cst.tile([128, 1], BF16)
            nc.sync.dma_start(out=onesc_sb[:], in_=onesc[:])
            onesr_sb = cst.tile([1, 128], BF16)
            nc.sync.dma_start(out=onesr_sb[:], in_=onesr[:])
            wo_sb = wts.tile([128, GQ * DT, 128], BF16)
            nc.sync.dma_start(out=wo_sb[:], in_=wo4[:])

            kT = big.tile([128, S], BF16)            # [HD, k]
            vN = big.tile([128, NKT, HD], BF16)      # [tok, kt, hd]
            qT = big.tile([128, GQ, S], BF16)        # [HD, h, q]
            attnT = big.tile([128, NKT, GQ * 128], BF16)  # [hd, qt, (j,q)]
            # P tiles, one per kt: [k-tok, h, q span]
            P_kt = [big.tile([128, GQ, S - kt * 128], BF16, name=f"P{kt}")
                    for kt in range(NKT)]

            # ---- K/V/Q projections: one PE-bound pass over streaming x ----
            with tc.tile_pool(name="ps_a", bufs=1, space="PSUM") as ps_a:
                # rope scratch in bf16: halves the rotate DMA and enables the
                # DVE 2x packed mode on the mul/add chain
                ktmp = rp.tile([128, S], BF16)
                krot = rp.tile([128, S], BF16)
                tmpa = rp.tile([128, S], BF16)
                tmpb = rp.tile([128, S], BF16)
                warm = ps_a.tile([128, 512], F32, tag="psk0", name="warm")
                for _ in range(24):
                    nc.tensor.matmul(warm[:], zt[:, :128], zt[:])
                psks = [ps_a.tile([128, 512], F32, tag=f"psk{i}",
                                  name=f"psk{i}") for i in range(2)]
                psqs = [ps_a.tile([128, S], F32, tag=f"psq{i}",
                                  name=f"psq{i}") for i in range(2)]
                psvs = [ps_a.tile([128, 128], F32, tag=f"psv{i}",
                                  name=f"psv{i}") for i in range(2)]

                def rope_q(j, psq):
                    qtmp = rp.tile([128, S], BF16, tag=f"qt{j % 2}",
                                   name=f"qtmp{j}")
                    qrot = rp.tile([128, S], BF16, tag=f"qr{j % 2}",
                                   name=f"qrot{j}")
                    nc.scalar.copy(qtmp[:], psq[:])
                    nc.sync.dma_start(out=qrot[0:64, :], in_=qtmp[64:128, :])
                    nc.sync.dma_start(out=qrot[64:128, :], in_=qtmp[0:64, :])
                    nc.vector.tensor_mul(qtmp[:], qtmp[:], cos_sb[:])
                    nc.vector.tensor_mul(qrot[:], qrot[:], sin_sb[:])
                    nc.vector.tensor_add(qT[:, j, :], qtmp[:], qrot[:])

                # pass 1 (DMA-overlapped): K + q heads 0,1 per x tile
                for d in range(DT):
                    for i, (c0, w) in enumerate(HQ):
                        nc.tensor.matmul(psks[i][:], wk_sb[:, d, :],
                                         xln_p[d // 2][:, d % 2, c0:c0 + w],
                                         start=(d == 0), stop=(d == DT - 1))
                    for j in range(2):
                        for c0, w in HQ:
                            nc.tensor.matmul(psqs[j][:, c0:c0 + w],
                                             wq_sb[:, j * DT + d, :],
                                             xln_p[d // 2][:, d % 2, c0:c0 + w],
                                             start=(d == 0), stop=(d == DT - 1))
                for i, (c0, w) in enumerate(HQ):
                    nc.scalar.copy(ktmp[:, c0:c0 + w], psks[i][:])
                nc.sync.dma_start(out=krot[0:64, :], in_=ktmp[64:128, :])
                nc.sync.dma_start(out=krot[64:128, :], in_=ktmp[0:64, :])
                nc.vector.tensor_mul(tmpa[:], ktmp[:], cos_sb[:])
                nc.vector.tensor_mul(tmpb[:], krot[:], sin_sb[:])
                nc.vector.tensor_add(kT[:], tmpa[:], tmpb[:])
                rope_q(0, psqs[0])
                rope_q(1, psqs[1])

                # pass 2 (from SBUF): q heads 2,3 reusing the q psum tiles
                for j in range(2, GQ):
                    psq = psqs[j - 2]
                    for d in range(DT):
                        for c0, w in HQ:
                            nc.tensor.matmul(psq[:, c0:c0 + w],
                                             wq_sb[:, j * DT + d, :],
                                             xln_p[d // 2][:, d % 2, c0:c0 + w],
                                             start=(d == 0), stop=(d == DT - 1))
                    rope_q(j, psq)

                # pass 3: V proj (t-major), ping-pong psum groups
                for kt in range(NKT):
                    psv = psvs[kt % 2]
                    for d in range(DT):
                        nc.tensor.matmul(psv[:],
                                         xln_p[d // 2][:, d % 2, kt * 128:(kt + 1) * 128],
                                         wv_sb[:, d, :],
                                         start=(d == 0), stop=(d == DT - 1))
                    nc.scalar.copy(vN[:, kt, :], psv[:])

            # ------- scores^T/exp interleaved with sums+PV+normalize -------
            with (
                tc.tile_pool(name="ps_s", bufs=3, space="PSUM") as ps_s,
                tc.tile_pool(name="ps_sum", bufs=2, space="PSUM") as ps_sum,
                tc.tile_pool(name="ps_pv", bufs=1, space="PSUM") as ps_pv,
                tc.tile_pool(name="ps_o", bufs=2, space="PSUM") as ps_o,
            ):
                def scores(kt):
                    span = S - kt * 128
                    for j in range(GQ):
                        for c0 in range(0, span, 512):
                            w = min(512, span - c0)
                            psS = ps_s.tile([128, 512], F32, tag="psS")
                            nc.tensor.matmul(
                                psS[:, :w],
                                kT[:, kt * 128:(kt + 1) * 128],
                                qT[:, j, kt * 128 + c0:kt * 128 + c0 + w])
                            nc.scalar.activation(P_kt[kt][:, j, c0:c0 + w],
                                                 psS[:, :w], AF.Exp)
                        # causal mask on the diagonal block: zero exp(s)
                        # above the diagonal (post-exp multiplicative mask;
                        # scores are O(5) so exp cannot overflow). bf16 SBUF
                        # in-place: DVE 4x packed mode, ~100ns — the sums
                        # matmuls wait on this, so it must be fast.
                        nc.vector.tensor_mul(P_kt[kt][:, j, 0:128],
                                             P_kt[kt][:, j, 0:128],
                                             trilm_sb[:])

                # per-(h,q) softmax denominators; reciprocal runs off the
                # critical path (ps_sum bufs=2 keeps the next sums group
                # from waiting on it)
                recips = [nrm.tile([1, GQ * 128], F32, name=f"recip{q}")
                          for q in range(NKT)]
                recips_bf = [nrm.tile([1, GQ * 128], BF16, name=f"recipb{q}")
                             for q in range(NKT)]

                def sums_pv(qt):
                    sums_ps = ps_sum.tile([1, GQ * 128], F32, tag="sums",
                                          name=f"sums{qt}")
                    pv_ps = ps_pv.tile([128, GQ * 128], F32, tag="pv")
                    for kt in range(qt + 1):
                        qoff = (qt - kt) * 128
                        nc.tensor.matmul(sums_ps[:], onesc_sb[:],
                                         P_kt[kt][:, :, qoff:qoff + 128],
                                         start=(kt == 0), stop=(kt == qt))
                    nc.vector.reciprocal_approx_fast(recips[qt][:],
                                                     sums_ps[:])
                    # bf16 copy so the broadcast matmul runs at bf16 rate
                    # (fp32 matmuls stream at 1/4 rate)
                    nc.vector.tensor_copy(recips_bf[qt][:], recips[qt][:])
                    for kt in range(qt + 1):
                        qoff = (qt - kt) * 128
                        nc.tensor.matmul(pv_ps[:], vN[:, kt, :],
                                         P_kt[kt][:, :, qoff:qoff + 128],
                                         start=(kt == 0), stop=(kt == qt))
                    nc.scalar.copy(attnT[:, qt, :], pv_ps[:])

                def norm_batch(qts):
                    for qt in qts:
                        # broadcast 1/sum along partitions via outer-product
                        # matmul into a psum slot shared with the sums tag
                        rbc_ps = ps_sum.tile([128, GQ * 128], F32, tag="sums",
                                             name=f"rbc{qt}")
                        nc.tensor.matmul(rbc_ps[:], onesr_sb[:],
                                         recips_bf[qt][:])
                        nc.vector.tensor_mul(attnT[:, qt, :], attnT[:, qt, :],
                                             rbc_ps[:])

                def outproj(ci):
                    # q chunk ci covers qt = 4*ci .. 4*ci+3
                    for d2 in range(DT):
                        pso = ps_o.tile([128, 512], F32, tag="pso")
                        for j in range(GQ):
                            nc.tensor.matmul(
                                pso[:], wo_sb[:, j * DT + d2, :],
                                attnT[:, 4 * ci:4 * (ci + 1),
                                      j * 128:(j + 1) * 128],
                                start=(j == 0), stop=(j == GQ - 1))
                        po_t = pop.tile([128, 512], BF16, tag="pot")
                        if d2 % 2 == 0:
                            nc.vector.tensor_copy(po_t[:], pso[:])
                        else:
                            nc.scalar.copy(po_t[:], pso[:])
                        nc.sync.dma_start(out=po[d2, :, 512 * ci:512 * (ci + 1)],
                                          in_=po_t[:])

                for kt in range(NKT):
                    scores(kt)
                    if kt >= 1:
                        sums_pv(kt - 1)
                    if kt == 5:
                        norm_batch(range(4))
                        outproj(0)
                sums_pv(NKT - 1)
                norm_batch(range(4, NKT))
                outproj(1)
    nc.compile()
    return nc


def _ln(x):
    mu = x.mean(-1, keepdims=True)
    var = x.var(-1, keepdims=True)
    return (x - mu) / np.sqrt(var + EPS)


def host_attn_inputs(x, cos, sin, ln1_w, w_qkv, w_out, n_cores=8):
    """Per-core input maps for build_attn. x [B,S,D] f32; cos/sin [S,HD]."""
    xln = (_ln(x) * ln1_w[None, None, :]).astype(np.float32)  # [B,S,D]
    wqkvT = w_qkv.T.astype(np.float32)                        # [D, 3072]
    scale = np.float32(1.0 / np.sqrt(HD))
    wq_all = wqkvT[:, :NH * HD] * scale
    wk_all = wqkvT[:, NH * HD:(NH + KVH) * HD]
    wv_all = wqkvT[:, (NH + KVH) * HD:]
    w_outT = w_out.T.astype(np.float32)                       # [O, D]
    sinp = sin.copy()
    sinp[:, :HD // 2] *= -1.0
    cosT = np.ascontiguousarray(cos.T).astype(BF)             # [HD, S]
    sinT = np.ascontiguousarray(sinp.T).astype(BF)

    kk = np.arange(128)[:, None]
    qq = np.arange(128)[None, :]
    trilm = (kk <= qq).astype(BF)                             # [k, q] keep mask
    onesc = np.ones((128, 1), BF)
    onesr = np.ones((1, 128), BF)

    xln_b = [np.ascontiguousarray(xln[b].T).astype(BF).reshape(DT, 128, S)
             for b in range(B)]
    maps = []
    for r in range(n_cores):
        b, g = divmod(r, KVH)
        wk_in = np.ascontiguousarray(
            wk_all[:, g * 128:(g + 1) * 128]
            .reshape(DT, 128, 128).transpose(1, 0, 2)).astype(BF)
        wv_in = np.ascontiguousarray(
            wv_all[:, g * 128:(g + 1) * 128]
            .reshape(DT, 128, 128).transpose(1, 0, 2)).astype(BF)
        wq_in = np.ascontiguousarray(
            wq_all[:, g * GQ * HD:(g + 1) * GQ * HD]
            .reshape(DT, 128, GQ, 128).transpose(1, 2, 0, 3)
            .reshape(128, GQ * DT, 128)).astype(BF)
        wo_in = np.ascontiguousarray(
            w_outT[g * GQ * HD:(g + 1) * GQ * HD]
            .reshape(GQ, 128, DT, 128).transpose(1, 0, 2, 3)
            .reshape(128, GQ * DT, 128)).astype(BF)
        maps.append({
            "xln": xln_b[b], "wk1": wk_in, "wv1": wv_in, "wq4": wq_in,
            "wo4": wo_in, "cosk": cosT, "sink": sinT, "trilm": trilm,
            "onesc": onesc, "onesr": onesr,
        })
    return maps


def assemble_attn_outputs(results, x, n_cores=8):
    """Sum per-core partial out-projections, add residual. Returns resid
    [B,S,D] f32."""
    attn = np.zeros((B, D, S), np.float32)
    for r in range(n_cores):
        b = r // KVH
        attn[b] += results[r]["po"].reshape(D, S).astype(np.float32)
    resid = x + attn.transpose(0, 2, 1)
    return resid


# ======================= MoE launch (expert parallel) =======================

def chunks(C):
    # free-dim chunks <=512 (PSUM bank), prefer fewest chunks all >=256
    if C <= 512:
        return [(0, C)]
    if C <= 1024:
        h = (C // 2 + 31) // 32 * 32
        return [(0, h), (h, C - h)]
    return [(0, 512), (512, 512), (1024, C - 1024)]


def build_moe(C, n_cores=8, _act=None):
    act = AF.Silu if _act is None else _act
    CH = chunks(C)
    nc = bacc.Bacc("TRN2", target_bir_lowering=False, debug=False,
                   num_devices=n_cores)
    xe = nc.dram_tensor("xe", [DT_, 128, C], BF16, kind="ExternalInput").ap()
    wg = nc.dram_tensor("wg", [FT, 128, DT_, 128], BF16,
                        kind="ExternalInput").ap()
    wu = nc.dram_tensor("wu", [FT, 128, DT_, 128], BF16,
                        kind="ExternalInput").ap()
    wd = nc.dram_tensor("wd", [DT_, 128, FT, 128], BF16,
                        kind="ExternalInput").ap()
    wec = nc.dram_tensor("wec", [1, C], F32, kind="ExternalInput").ap()
    ye = nc.dram_tensor("ye", [DT_, 128, C], BF16, kind="ExternalOutput").ap()

    with tile.TileContext(nc) as tc:
        with (
            tc.tile_pool(name="res", bufs=1) as res,
            tc.tile_pool(name="wp", bufs=3) as wp,
            tc.tile_pool(name="sg", bufs=3) as sgp,
            tc.tile_pool(name="yo", bufs=3) as yop,
        ):
            # startup: DMAs issued in exact consumption order of the f=0
            # d-loop (4-d weight chunks interleaved with x tiles), so the
            # first matmul needs only ~0.5 MB. HAM warm-up dummies (below)
            # cover the wait at full duty so real matmuls start warm.
            zt = res.tile([128, 512], BF16)
            nc.gpsimd.memset(zt[:], 0.0)
            xsb_p = [res.tile([128, 2, C], BF16, name=f"xep{i}")
                     for i in range(DT_ // 2)]
            wpre = []
            wgt_p = wp.tile([128, DT_, 128], BF16, tag="wg")
            wut_p = wp.tile([128, DT_, 128], BF16, tag="wu")
            # first tiles on separate engine queues (parallel HWDGE
            # descriptor generation during the cold start)
            nc.scalar.dma_start(out=wgt_p[:, :8, :], in_=wg[0, :, :8, :])
            nc.gpsimd.dma_start(out=wut_p[:, :8, :], in_=wu[0, :, :8, :])
            nc.sync.dma_start(
                out=xsb_p[0][:], in_=xe[0:2].rearrange("d p c -> p d c"))
            nc.gpsimd.dma_start(
                out=xsb_p[1][:], in_=xe[2:4].rearrange("d p c -> p d c"))
            for i in (2, 3):
                nc.sync.dma_start(
                    out=xsb_p[i][:],
                    in_=xe[2 * i:2 * i + 2].rearrange("d p c -> p d c"))
            nc.sync.dma_start(out=wgt_p[:, 8:, :], in_=wg[0, :, 8:, :])
            nc.sync.dma_start(out=wut_p[:, 8:, :], in_=wu[0, :, 8:, :])
            for i in range(4, DT_ // 2):
                nc.sync.dma_start(
                    out=xsb_p[i][:],
                    in_=xe[2 * i:2 * i + 2].rearrange("d p c -> p d c"))
            wpre.append((wgt_p, wut_p))
            wgt_p = wp.tile([128, DT_, 128], BF16, tag="wg")
            nc.sync.dma_start(out=wgt_p[:], in_=wg[1])
            wut_p = wp.tile([128, DT_, 128], BF16, tag="wu")
            nc.sync.dma_start(out=wut_p[:], in_=wu[1])
            wpre.append((wgt_p, wut_p))
            webc = res.tile([128, C], F32)
            nc.sync.dma_start(
                out=webc[:],
                in_=bass.AP(tensor=wec.tensor, offset=wec.offset,
                            ap=[[0, 128], [1, C]]),
            )
            mT = res.tile([128, FT, C], BF16)

            # --- gate/up + silu*u -> mT ---
            with (
                tc.tile_pool(name="psgu", bufs=1, space="PSUM") as psg,
                tc.tile_pool(name="psy", bufs=2, space="PSUM") as psy,
            ):
                warm = psg.tile([128, 512], F32, tag="pg0", name="warm")
                for _ in range(16):
                    nc.tensor.matmul(warm[:], zt[:, :128], zt[:])
                for f in range(FT):
                    pgs = [psg.tile([128, w], F32, name=f"pg{ci}", tag=f"pg{ci}")
                           for ci, (_, w) in enumerate(CH)]
                    pus = [psg.tile([128, w], F32, name=f"pu{ci}", tag=f"pu{ci}")
                           for ci, (_, w) in enumerate(CH)]
                    if f < 2:
                        wgt, wut = wpre[f]
                    else:
                        wgt = wp.tile([128, DT_, 128], BF16, tag="wg")
                        nc.sync.dma_start(out=wgt[:], in_=wg[f])
                        wut = wp.tile([128, DT_, 128], BF16, tag="wu")
                        nc.sync.dma_start(out=wut[:], in_=wu[f])
                    for d in range(DT_):
                        for ci, (c0, w) in enumerate(CH):
                            nc.tensor.matmul(pgs[ci][:], wgt[:, d, :],
                                             xsb_p[d // 2][:, d % 2, c0:c0 + w],
                                             start=(d == 0), stop=(d == DT_ - 1))
                        for ci, (c0, w) in enumerate(CH):
                            nc.tensor.matmul(pus[ci][:], wut[:, d, :],
                                             xsb_p[d // 2][:, d % 2, c0:c0 + w],
                                             start=(d == 0), stop=(d == DT_ - 1))
                    for ci, (c0, w) in enumerate(CH):
                        sg = sgp.tile([128, 512], F32, tag="sg")
                        nc.scalar.activation(sg[:, :w], pgs[ci][:], act)
                        nc.vector.tensor_mul(mT[:, f, c0:c0 + w], sg[:, :w],
                                             pus[ci][:])

                # --- down + combine-weight scale -> ye ---
                for d2 in range(DT_):
                    pys = [psy.tile([128, w], F32, name=f"py{ci}", tag=f"py{ci}")
                           for ci, (_, w) in enumerate(CH)]
                    wdt = wp.tile([128, FT, 128], BF16, tag="wd")
                    nc.sync.dma_start(out=wdt[:], in_=wd[d2])
                    for f in range(FT):
                        for ci, (c0, w) in enumerate(CH):
                            nc.tensor.matmul(pys[ci][:], wdt[:, f, :],
                                             mT[:, f, c0:c0 + w],
                                             start=(f == 0), stop=(f == FT - 1))
                    for ci, (c0, w) in enumerate(CH):
                        yt = yop.tile([128, 512], BF16, tag="yt")
                        nc.vector.tensor_mul(yt[:, :w], pys[ci][:],
                                             webc[:, c0:c0 + w])
                        nc.sync.dma_start(out=ye[d2, :, c0:c0 + w], in_=yt[:, :w])
    nc.compile()
    return nc


def host_moe_inputs(h2T_full, assign, aw, C, w_gate, w_up, w_down):
    """Per-core input maps. h2T_full [D, T] f32; assign/aw lists per expert."""
    maps = []
    for e in range(E):
        n = len(assign[e])
        assert n <= C, f"expert {e} count {n} > capacity {C}"
        xeT = np.zeros((MD, C), BF)
        xeT[:, :n] = h2T_full[:, assign[e]].astype(BF)
        wec = np.zeros((1, C), np.float32)
        wec[0, :n] = aw[e]
        maps.append({
            "xe": np.ascontiguousarray(xeT.reshape(DT_, 128, C)),
            "wg": np.ascontiguousarray(
                w_gate[e].reshape(DT_, 128, FT, 128)
                .transpose(2, 1, 0, 3)).astype(BF),
            "wu": np.ascontiguousarray(
                w_up[e].reshape(DT_, 128, FT, 128)
                .transpose(2, 1, 0, 3)).astype(BF),
            "wd": np.ascontiguousarray(
                w_down[e].reshape(FT, 128, DT_, 128)
                .transpose(2, 1, 0, 3)).astype(BF),
            "wec": wec,
        })
    return maps


# ======================= top-level kernel =======================
_cache = {}


def _routing(logits):
    lm = logits.max(1, keepdims=True)
    p = np.exp(logits - lm)
    p /= p.sum(1, keepdims=True)
    top_e = np.argsort(-p, 1)[:, :K_TOP]
    top_w = np.take_along_axis(p, top_e, 1)
    top_w = top_w / np.abs(top_w).sum(1, keepdims=True)
    flat_e = top_e.ravel()
    flat_t = np.repeat(np.arange(logits.shape[0]), K_TOP)
    flat_w = top_w.ravel()
    assign = [flat_t[flat_e == e] for e in range(E)]
    aw = [flat_w[flat_e == e] for e in range(E)]
    return assign, aw


def kernel(hidden_states, cos, sin, ln1_w, ln2_w, w_qkv, w_out,
           w_router, w_gate, w_up, w_down):
    hidden_states = np.asarray(hidden_states, np.float32)
    cos = np.asarray(cos, np.float32)
    sin = np.asarray(sin, np.float32)
    ln1_w = np.asarray(ln1_w, np.float32)
    ln2_w = np.asarray(ln2_w, np.float32)
    w_qkv = np.asarray(w_qkv, np.float32)
    w_out = np.asarray(w_out, np.float32)
    w_router = np.asarray(w_router, np.float32)
    w_gate = np.asarray(w_gate, np.float32)
    w_up = np.asarray(w_up, np.float32)
    w_down = np.asarray(w_down, np.float32)

    if "attn" not in _cache:
        _cache["attn"] = build_attn()
    maps = host_attn_inputs(hidden_states, cos, sin, ln1_w, w_qkv, w_out)
    res1 = run_bass_kernel_spmd(_cache["attn"], maps, list(range(8)))
    resid = assemble_attn_outputs(res1.results, hidden_states)

    h2 = (_ln(resid) * ln2_w[None, None, :]).reshape(-1, D)   # [T, D]
    logits = h2 @ w_router.T
    assign, aw = _routing(logits)
    counts = [len(a) for a in assign]
    C = max(256, (max(counts) + 1) // 2 * 2)

    if ("moe", C) not in _cache:
        _cache[("moe", C)] = build_moe(C)
    h2T = np.ascontiguousarray(h2.T)                          # [D, T]
    maps2 = host_moe_inputs(h2T, assign, aw, C, w_gate, w_up, w_down)
    res2 = run_bass_kernel_spmd(_cache[("moe", C)], maps2, list(range(8)))

    T = B * S
    out_full = np.zeros((T, MD), np.float32)
    for e in range(E):
        ye = res2.results[e]["ye"].reshape(MD, C).astype(np.float32)
        n = counts[e]
        out_full[assign[e]] += ye[:, :n].T

    out = out_full.reshape(B, S, D)
    return out, resid
